# revision 14
# baseline (speedup 1.0000x reference)
"""AttentionLayer pooling kernel for 8 Trainium2 NeuronCores.

Reference computation (per example b):
    squish = tanh(xes @ weight + bias)          # [T, H]
    attn   = (squish @ proj)[:, 0] / 16         # [T]
    attn   = where(mask, attn, -1e6)
    w      = softmax(attn)                      # over T
    out    = w @ xes                            # [H]

Sharding: data-parallel over batch. B=32 examples over 8 cores = 4/core.
Each core reads its 4x4096x256 xes shard (bf16), computes scores and the
weighted sum fully locally, writes [4, 256] f32. No collectives.

Per-core dataflow (T split into 128 chunks of 128 rows; gg = e*32 + g):
  PE:  S[t,j] = sum_h XT[h,t].T @ W[h,j]   (XT stationary, chunked over h)
  ACT: squish = tanh(S)  (PSUM -> SBUF bf16)
  DVE: scores[t] = sum_j squish[t,j]*proj[j]    (tensor_tensor_reduce)
  ACT: e = exp(scores/16);  DVE: e *= maskT, rowsum
  PE:  denom = ones.T @ rowsum;  y_ps = sum_g e_col.T @ Xnat_chunk
  DVE: y = y_ps / denom

Softmax skips max-subtraction: |scores/16| <= 0.1 so exp never overflows,
and masked entries are zeroed by the mask multiply (identical to the
reference where exp(-1e6 - max) underflows to exactly 0.0 in f32).
"""

import numpy as np
import ml_dtypes

import concourse.bacc as bacc
import concourse.mybir as mybir
import concourse.tile as tile
from concourse import bass_utils

BF16 = mybir.dt.bfloat16
FP8 = mybir.dt.float8e4
F32 = mybir.dt.float32
bf16 = ml_dtypes.bfloat16
fp8 = mybir.dt.np(mybir.dt.float8e4)
USE_FP8 = True

B, T, H = 32, 4096, 256
NCORES = 8
BE = B // NCORES            # examples per core
NCH = BE * T // 128         # 128-row chunks per core
CPE = T // 128              # chunks per example (32)
NTILE = 8                   # 512-row tiles per example
SUB = 4                     # 128-row sub-chunks per tile
TT = 512                    # t-columns per tile

_compiled = {}


def _build(has_bias: bool):
    nc = bacc.Bacc("TRN2", target_bir_lowering=False, debug=False)

    xn_d = nc.dram_tensor("xn", [128, NCH * H], BF16, kind="ExternalInput")
    XDT = FP8 if USE_FP8 else BF16
    xt_d = nc.dram_tensor("xt", [BE * 2, 128, T], XDT, kind="ExternalInput")
    w_d = nc.dram_tensor("w", [2, 128, H], XDT, kind="ExternalInput")
    pj_d = nc.dram_tensor("pj", [128, 2], F32, kind="ExternalInput")
    mt_d = nc.dram_tensor("mt", [128, NCH], BF16, kind="ExternalInput")
    if has_bias:
        bias_d = nc.dram_tensor("bias", [128, 2], F32, kind="ExternalInput")
    y_d = nc.dram_tensor("y", [BE, H], F32, kind="ExternalOutput")

    Tanh = mybir.ActivationFunctionType.Tanh
    Exp = mybir.ActivationFunctionType.Exp

    with tile.TileContext(nc) as tc:
        with (
            tc.tile_pool(name="res", bufs=1) as res,
            tc.tile_pool(name="sqp", bufs=6) as sqp,
            tc.tile_pool(name="scrp", bufs=6) as scrp,
            tc.tile_pool(name="exp", bufs=2) as exp_pool,
            tc.tile_pool(name="sps", bufs=2, space="PSUM") as sps,
            tc.tile_pool(name="scps", bufs=2, space="PSUM") as scps,
            tc.tile_pool(name="yps", bufs=1, space="PSUM") as yps,
            tc.tile_pool(name="ssum", bufs=1, space="PSUM") as ssum,
        ):
            xn_sb = res.tile([128, NCH * H], BF16)
            xt_sb = res.tile([128, BE * 2 * T], XDT)
            w_sb = res.tile([128, 2 * H], XDT)
            pj_sb = res.tile([128, 2], F32)
            mt_sb = res.tile([128, NCH], BF16)
            ones = res.tile([128, 1], F32)
            ones_bf = res.tile([128, 1], BF16)
            e_m = res.tile([128, NCH], BF16)
            y_sb = res.tile([1, BE * H], F32)
            if has_bias:
                bias_sb = res.tile([128, 2], F32)
                nc.scalar.dma_start(bias_sb[:], bias_d.ap())

            warm_rhs = res.tile([128, 256], BF16)
            warm_lhs = res.tile([128, 128], BF16)

            nc.vector.memset(ones[:], 1.0)
            nc.vector.memset(ones_bf[:], 1.0)
            nc.vector.memset(warm_rhs[:], 0.5)
            nc.vector.memset(warm_lhs[:], 0.5)

            # Small params on the ACT HWDGE ring (idle at start); big xt
            # pieces on the SP HWDGE ring; xn on the gpsimd SWDGE ring.
            # Keeping the three streams on separate issuing engines avoids
            # the ~600ns-per-issue serialization that throttled the ramp.
            nc.scalar.dma_start(w_sb[:, 0:H], w_d.ap()[0])
            nc.scalar.dma_start(w_sb[:, H : 2 * H], w_d.ap()[1])
            nc.scalar.dma_start(pj_sb[:], pj_d.ap())
            nc.scalar.dma_start(mt_sb[:], mt_d.ap())

            # PE warmup: ~18 chained matmuls (~3.5us cold) into the yps bank
            # keep the PE busy through the DMA ramp so HAM unthrottles to
            # 2.4 GHz right as the first real S-matmul issues.
            # Full-array (K=128, M=128) matmuls — HAM watches MAC-array
            # activity, so thin warmups don't unthrottle the clock.
            warm_ps = sps.tile([128, 2 * TT], F32, tag="s_ps")
            NWARM = 16
            for k in range(NWARM):
                nc.tensor.matmul(
                    warm_ps[:, 0:256],
                    warm_lhs[:],
                    warm_rhs[:],
                    start=(k == 0),
                    stop=(k == NWARM - 1),
                    skip_group_check=True,
                )

            # All big inputs on ONE sync HWDGE queue, interleaved in the
            # order compute consumes them (a single FIFO queue gives each
            # piece the full HBM rate; two concurrent queues share it and
            # SWDGE starves HWDGE). Need-order: xt_e0, xt_e1, xn_e0,
            # xt_e2, xn_e1, xt_e3, xn_e2, xn_e3.
            def xt_load(e, npieces=1):
                step = T // npieces
                for piece in range(npieces):
                    lo, hi = piece * step, (piece + 1) * step
                    for hc in range(2):
                        base = (e * 2 + hc) * T
                        nc.sync.dma_start(
                            xt_sb[:, base + lo : base + hi],
                            xt_d.ap()[e * 2 + hc][:, lo:hi],
                        )

            def xn_load(e):
                g0 = e * CPE
                nc.sync.dma_start(
                    xn_sb[:, g0 * H : (g0 + CPE) * H],
                    xn_d.ap()[:, g0 * H : (g0 + CPE) * H],
                )

            # e0's hc1 half rides the scalar HWDGE queue in parallel with
            # hc0 on sync, halving time-to-first-compute.
            for piece in range(2):
                lo, hi = piece * (T // 2), (piece + 1) * (T // 2)
                nc.sync.dma_start(
                    xt_sb[:, lo:hi], xt_d.ap()[0][:, lo:hi]
                )
                nc.scalar.dma_start(
                    xt_sb[:, T + lo : T + hi], xt_d.ap()[1][:, lo:hi]
                )
            xt_load(1)
            xn_load(0)
            xt_load(2)
            xn_load(1)
            xt_load(3)
            xn_load(2)
            xn_load(3)

            madds = {}
            sc_tiles = {}

            w3 = w_sb[:].rearrange("p (c j) -> p c j", c=2)
            xt4 = xt_sb[:].rearrange("p (e c t) -> p e c t", e=BE, c=2)

            def main_tile(e, i):
                # S^T tile: partition = j (2 blocks of 128), free = t (512).
                s_ps = sps.tile([128, 2 * TT], F32)
                for jb in range(2):
                    if USE_FP8:
                        # Both K-subtiles (h chunks) in one DoubleRow matmul.
                        nc.tensor.matmul(
                            s_ps[:, jb * TT : (jb + 1) * TT],
                            w3[:, :, jb * 128 : (jb + 1) * 128],
                            xt4[:, e, :, i * TT : (i + 1) * TT],
                            start=True,
                            stop=True,
                            perf_mode=mybir.MatmulPerfMode.DoubleRow,
                        )
                    else:
                        for hc in range(2):
                            nc.tensor.matmul(
                                s_ps[:, jb * TT : (jb + 1) * TT],
                                w_sb[:, hc * H + jb * 128 : hc * H + (jb + 1) * 128],
                                xt_sb[:, (e * 2 + hc) * T + i * TT : (e * 2 + hc) * T + (i + 1) * TT],
                                start=(hc == 0),
                                stop=(hc == 1),
                            )
                sq = sqp.tile([128, 2 * TT], BF16)
                if has_bias:
                    for jb in range(2):
                        nc.scalar.activation(
                            sq[:, jb * TT : (jb + 1) * TT],
                            s_ps[:, jb * TT : (jb + 1) * TT],
                            Tanh,
                            bias=bias_sb[:, jb : jb + 1],
                        )
                else:
                    nc.scalar.activation(sq[:], s_ps[:], Tanh)

                # madd[j, t] = sq0*proj0 + sq1*proj1  (per-partition scalars, 4x mode)
                m0 = scrp.tile([128, TT], BF16, tag="m0")
                madd = scrp.tile([128, TT], BF16, tag="madd")
                nc.vector.tensor_scalar_mul(m0[:], sq[:, 0:TT], pj_sb[:, 0:1])
                nc.vector.tensor_scalar_mul(madd[:], sq[:, TT : 2 * TT], pj_sb[:, 1:2])
                nc.vector.tensor_add(madd[:], madd[:], m0[:])
                madds[(e, i)] = madd

            def scores_mm(e, i):
                # scores[t] = sum_j madd[j, t] via PE: 4 chunks of 128 t-cols,
                # accumulated straight into the example's PSUM score tile.
                madd = madds.pop((e, i))
                if i == 0:
                    sc_tiles[e] = scps.tile([128, CPE], F32, name="sc", tag="sc")
                sc_ps = sc_tiles[e]
                for c in range(SUB):
                    nc.tensor.matmul(
                        sc_ps[:, i * SUB + c : i * SUB + c + 1],
                        madd[:, c * 128 : (c + 1) * 128],
                        ones_bf[:],
                        start=True,
                        stop=True,
                    )

            def final_head_full(e, y_tiles):
                # Full per-example head: exp+mask all chunks, rowsum,
                # denominator matmul, reciprocal. Runs once, early, so the
                # y-matmuls can spread evenly over the next example's tiles.
                c0, c1 = e * CPE, (e + 1) * CPE
                e_bf = exp_pool.tile([128, CPE], BF16, name="e_bf", tag="e_bf")
                sc_ps = sc_tiles.pop(e)
                nc.scalar.activation(e_bf[:], sc_ps[:], Exp, scale=1.0 / 16.0)
                nc.vector.tensor_mul(e_m[:, c0:c1], e_bf[:], mt_sb[:, c0:c1])
                rowsum = exp_pool.tile([128, 1], F32)
                nc.vector.reduce_sum(
                    out=rowsum[:], in_=e_m[:, c0:c1], axis=mybir.AxisListType.X
                )
                s1 = ssum.tile([1, 1], F32)
                nc.tensor.matmul(s1[:], rowsum[:], ones[:], start=True, stop=True)
                sinv = exp_pool.tile([1, 1], F32)
                nc.vector.reciprocal(sinv[:], s1[:])
                y_ps = yps.tile([1, H], F32)
                y_tiles[e] = (y_ps, sinv)

            def final_mms_sp(e, y_tiles, glo, ghi):
                y_ps, _ = y_tiles[e]
                for g in range(glo, ghi):
                    gg = e * CPE + g
                    nc.tensor.matmul(
                        y_ps[:],
                        e_m[:, gg : gg + 1],
                        xn_sb[:, gg * H : (gg + 1) * H],
                        start=(g == 0),
                        stop=(g == CPE - 1),
                        skip_group_check=True,
                    )

            def final_fin(e, y_tiles):
                y_ps, sinv = y_tiles.pop(e)
                nc.vector.tensor_scalar_mul(y_sb[:, e * H : (e + 1) * H], y_ps[:], sinv[:])
                nc.sync.dma_start(y_d.ap()[e : e + 1, :], y_sb[:, e * H : (e + 1) * H])

            def final_head(e, y_tiles, ghi):
                # exp/mask for score cols [0, ghi) + y psum allocation.
                c0 = e * CPE
                e_bf = exp_pool.tile([128, CPE], BF16, name="e_bf", tag="e_bf")
                sc_ps = sc_tiles[e]
                nc.scalar.activation(
                    e_bf[:, 0:ghi], sc_ps[:, 0:ghi], Exp, scale=1.0 / 16.0
                )
                nc.vector.tensor_mul(
                    e_m[:, c0 : c0 + ghi], e_bf[:, 0:ghi], mt_sb[:, c0 : c0 + ghi]
                )
                y_ps = yps.tile([1, H], F32)
                y_tiles[e] = (y_ps, e_bf)

            def final_mms(e, y_tiles, glo, ghi):
                y_ps, _ = y_tiles[e]
                for g in range(glo, ghi):
                    gg = e * CPE + g
                    nc.tensor.matmul(
                        y_ps[:],
                        e_m[:, gg : gg + 1],
                        xn_sb[:, gg * H : (gg + 1) * H],
                        start=(g == 0),
                        stop=False,
                        skip_group_check=True,
                    )

            def final_tail(e, y_tiles, glo):
                c0, c1 = e * CPE, (e + 1) * CPE
                y_ps, e_bf = y_tiles.pop(e)
                sc_ps = sc_tiles.pop(e)
                if glo < CPE:
                    nc.scalar.activation(
                        e_bf[:, glo:CPE], sc_ps[:, glo:CPE], Exp, scale=1.0 / 16.0
                    )
                    nc.vector.tensor_mul(
                        e_m[:, c0 + glo : c1], e_bf[:, glo:CPE], mt_sb[:, c0 + glo : c1]
                    )
                rowsum = exp_pool.tile([128, 1], F32)
                nc.vector.reduce_sum(
                    out=rowsum[:], in_=e_m[:, c0:c1], axis=mybir.AxisListType.X
                )
                s1 = ssum.tile([1, 1], F32)
                nc.tensor.matmul(s1[:], rowsum[:], ones[:], start=True, stop=True)
                sinv = exp_pool.tile([1, 1], F32)
                nc.vector.reciprocal(sinv[:], s1[:])
                gfrom = min(glo, CPE - 1)
                for g in range(gfrom, CPE):
                    gg = e * CPE + g
                    nc.tensor.matmul(
                        y_ps[:],
                        e_m[:, gg : gg + 1],
                        xn_sb[:, gg * H : (gg + 1) * H],
                        start=False,
                        stop=(g == CPE - 1),
                        skip_group_check=True,
                    )
                nc.vector.tensor_scalar_mul(y_sb[:, e * H : (e + 1) * H], y_ps[:], sinv[:])
                nc.sync.dma_start(y_d.ap()[e : e + 1, :], y_sb[:, e * H : (e + 1) * H])

            def final_ex(e, y_tiles):
                final_head(e, y_tiles, CPE)
                final_tail(e, y_tiles, CPE)

            # Software-pipelined emission: scores for tile k are emitted two
            # tiles later so PE never stalls on ACT/DVE of tile k. The
            # previous example's weighted-sum matmuls spread evenly over
            # tiles i=2..7 so no tile is PE-bound.
            pending = []
            y_tiles = {}
            for e in range(BE):
                for i in range(NTILE):
                    main_tile(e, i)
                    pending.append((e, i))
                    if len(pending) > 2:
                        scores_mm(*pending.pop(0))
                    if e > 0:
                        if i == 2:
                            final_head_full(e - 1, y_tiles)
                            final_mms_sp(e - 1, y_tiles, 0, 2)
                        elif 3 <= i <= 7:
                            final_mms_sp(
                                e - 1, y_tiles, 2 + (i - 3) * 6, 2 + (i - 2) * 6
                            )
                        if i == 7:
                            final_fin(e - 1, y_tiles)
            # Last example: overlap 7 tiles' weighted sums with the last
            # tile's score chain, then finish with the final 4 chunks.
            eL = BE - 1
            scores_mm(*pending.pop(0))
            final_head(eL, y_tiles, (NTILE - 1) * SUB)
            final_mms(eL, y_tiles, 0, (NTILE - 1) * SUB)
            scores_mm(*pending.pop(0))
            final_tail(eL, y_tiles, (NTILE - 1) * SUB)

    nc.compile()
    return nc


def _get(has_bias: bool):
    if has_bias not in _compiled:
        _compiled[has_bias] = _build(has_bias)
    return _compiled[has_bias]


def make_in_maps(xes, mask, weight, bias, proj):
    xes = np.asarray(xes, dtype=np.float32)
    mask = np.asarray(mask)
    weight = np.asarray(weight, dtype=np.float32)
    bias = np.asarray(bias, dtype=np.float32)
    proj = np.asarray(proj, dtype=np.float32)

    has_bias = bool(np.any(bias))
    w2 = np.ascontiguousarray(weight.astype(fp8 if USE_FP8 else bf16).reshape(2, 128, H))
    pj = np.ascontiguousarray(proj[:, 0].astype(np.float32).reshape(2, 128).T)
    in_maps = []
    for c in range(NCORES):
        xs = xes[c * BE : (c + 1) * BE]
        xn = np.ascontiguousarray(
            xs.astype(bf16).reshape(NCH, 128, H).transpose(1, 0, 2)
        ).reshape(128, NCH * H)
        xdt = fp8 if USE_FP8 else bf16
        xt = np.ascontiguousarray(xs.transpose(0, 2, 1)).astype(xdt).reshape(
            BE * 2, 128, T
        )
        mt = np.ascontiguousarray(
            mask[c * BE : (c + 1) * BE]
            .astype(np.float32)
            .reshape(BE, CPE, 128)
            .transpose(2, 0, 1)
            .reshape(128, NCH)
        ).astype(bf16)
        m = {"xn": xn, "xt": xt, "w": w2, "pj": pj, "mt": mt}
        if has_bias:
            m["bias"] = np.ascontiguousarray(bias.reshape(2, 128).T)
        in_maps.append(m)
    return in_maps, has_bias


def _run(in_maps, has_bias, **kwargs):
    nc = _get(has_bias)
    return bass_utils.run_bass_kernel_spmd(
        nc, in_maps, core_ids=list(range(NCORES)), **kwargs
    )


def kernel(xes, mask, weight, bias, proj):
    in_maps, has_bias = make_in_maps(xes, mask, weight, bias, proj)
    res = _run(in_maps, has_bias)
    return np.concatenate([res.results[c]["y"] for c in range(NCORES)], axis=0)



# revision 15
# speedup vs baseline: 1.2650x; 1.2650x over previous
"""AttentionLayer pooling kernel for 8 Trainium2 NeuronCores.

Reference computation (per example b):
    squish = tanh(xes @ weight + bias)          # [T, H]
    attn   = (squish @ proj)[:, 0] / 16         # [T]
    attn   = where(mask, attn, -1e6)
    w      = softmax(attn)                      # over T
    out    = w @ xes                            # [H]

Sharding: data-parallel over batch. B=32 examples over 8 cores = 4/core.
Each core reads its 4x4096x256 xes shard (bf16), computes scores and the
weighted sum fully locally, writes [4, 256] f32. No collectives.

Per-core dataflow (T split into 128 chunks of 128 rows; gg = e*32 + g):
  PE:  S[t,j] = sum_h XT[h,t].T @ W[h,j]   (XT stationary, chunked over h)
  ACT: squish = tanh(S)  (PSUM -> SBUF bf16)
  DVE: scores[t] = sum_j squish[t,j]*proj[j]    (tensor_tensor_reduce)
  ACT: e = exp(scores/16);  DVE: e *= maskT, rowsum
  PE:  denom = ones.T @ rowsum;  y_ps = sum_g e_col.T @ Xnat_chunk
  DVE: y = y_ps / denom

Softmax skips max-subtraction: |scores/16| <= 0.1 so exp never overflows,
and masked entries are zeroed by the mask multiply (identical to the
reference where exp(-1e6 - max) underflows to exactly 0.0 in f32).
"""

import numpy as np
import ml_dtypes

import concourse.bacc as bacc
import concourse.mybir as mybir
import concourse.tile as tile
from concourse import bass_utils

BF16 = mybir.dt.bfloat16
FP8 = mybir.dt.float8e4
F32 = mybir.dt.float32
bf16 = ml_dtypes.bfloat16
fp8 = mybir.dt.np(mybir.dt.float8e4)
USE_FP8 = True

B, T, H = 32, 4096, 256
NCORES = 8
BE = B // NCORES            # examples per core
NCH = BE * T // 128         # 128-row chunks per core
CPE = T // 128              # chunks per example (32)
NTILE = 8                   # 512-row tiles per example
SUB = 4                     # 128-row sub-chunks per tile
TT = 512                    # t-columns per tile

_compiled = {}


def _build(has_bias: bool):
    nc = bacc.Bacc("TRN2", target_bir_lowering=False, debug=False)

    xn_d = nc.dram_tensor("xn", [128, NCH * H], BF16, kind="ExternalInput")
    XDT = FP8 if USE_FP8 else BF16
    xt_d = nc.dram_tensor("xt", [BE * 2, 128, T], XDT, kind="ExternalInput")
    w_d = nc.dram_tensor("w", [2, 128, H], XDT, kind="ExternalInput")
    pj_d = nc.dram_tensor("pj", [128, 2], F32, kind="ExternalInput")
    mt_d = nc.dram_tensor("mt", [128, NCH], BF16, kind="ExternalInput")
    if has_bias:
        bias_d = nc.dram_tensor("bias", [128, 2], F32, kind="ExternalInput")
    y_d = nc.dram_tensor("y", [BE, H], F32, kind="ExternalOutput")

    Tanh = mybir.ActivationFunctionType.Tanh
    Exp = mybir.ActivationFunctionType.Exp

    with tile.TileContext(nc) as tc:
        with (
            tc.tile_pool(name="res", bufs=1) as res,
            tc.tile_pool(name="sqp", bufs=6) as sqp,
            tc.tile_pool(name="scrp", bufs=6) as scrp,
            tc.tile_pool(name="exp", bufs=2) as exp_pool,
            tc.tile_pool(name="sps", bufs=2, space="PSUM") as sps,
            tc.tile_pool(name="scps", bufs=2, space="PSUM") as scps,
            tc.tile_pool(name="yps", bufs=1, space="PSUM") as yps,
            tc.tile_pool(name="ssum", bufs=1, space="PSUM") as ssum,
        ):
            xn_sb = res.tile([128, NCH * H], BF16)
            xt_sb = res.tile([128, BE * 2 * T], XDT)
            w_sb = res.tile([128, 2 * H], XDT)
            pj_sb = res.tile([128, 2], F32)
            mt_sb = res.tile([128, NCH], BF16)
            ones = res.tile([128, 1], F32)
            ones_bf = res.tile([128, 1], BF16)
            e_m = res.tile([128, NCH], BF16)
            y_sb = res.tile([1, BE * H], F32)
            if has_bias:
                bias_sb = res.tile([128, 2], F32)
                nc.scalar.dma_start(bias_sb[:], bias_d.ap())

            warm_rhs = res.tile([128, 256], BF16)
            warm_lhs = res.tile([128, 128], BF16)

            nc.vector.memset(ones[:], 1.0)
            nc.vector.memset(ones_bf[:], 1.0)
            nc.vector.memset(warm_rhs[:], 0.5)
            nc.vector.memset(warm_lhs[:], 0.5)

            # Small params on the ACT HWDGE ring (idle at start); big xt
            # pieces on the SP HWDGE ring; xn on the gpsimd SWDGE ring.
            # Keeping the three streams on separate issuing engines avoids
            # the ~600ns-per-issue serialization that throttled the ramp.
            nc.scalar.dma_start(w_sb[:, 0:H], w_d.ap()[0])
            nc.scalar.dma_start(w_sb[:, H : 2 * H], w_d.ap()[1])
            nc.scalar.dma_start(pj_sb[:], pj_d.ap())
            nc.scalar.dma_start(mt_sb[:], mt_d.ap())

            # PE warmup: ~18 chained matmuls (~3.5us cold) into the yps bank
            # keep the PE busy through the DMA ramp so HAM unthrottles to
            # 2.4 GHz right as the first real S-matmul issues.
            # Full-array (K=128, M=128) matmuls — HAM watches MAC-array
            # activity, so thin warmups don't unthrottle the clock.
            warm_ps = sps.tile([128, 2 * TT], F32, tag="s_ps")
            NWARM = 16
            for k in range(NWARM):
                nc.tensor.matmul(
                    warm_ps[:, 0:256],
                    warm_lhs[:],
                    warm_rhs[:],
                    start=(k == 0),
                    stop=(k == NWARM - 1),
                    skip_group_check=True,
                )

            # All big inputs on ONE sync HWDGE queue, interleaved in the
            # order compute consumes them (a single FIFO queue gives each
            # piece the full HBM rate; two concurrent queues share it and
            # SWDGE starves HWDGE). Need-order: xt_e0, xt_e1, xn_e0,
            # xt_e2, xn_e1, xt_e3, xn_e2, xn_e3.
            def xt_load(e, npieces=1):
                step = T // npieces
                for piece in range(npieces):
                    lo, hi = piece * step, (piece + 1) * step
                    for hc in range(2):
                        base = (e * 2 + hc) * T
                        nc.sync.dma_start(
                            xt_sb[:, base + lo : base + hi],
                            xt_d.ap()[e * 2 + hc][:, lo:hi],
                        )

            def xn_load(e):
                g0 = e * CPE
                nc.sync.dma_start(
                    xn_sb[:, g0 * H : (g0 + CPE) * H],
                    xn_d.ap()[:, g0 * H : (g0 + CPE) * H],
                )

            xt_load(0, npieces=2)
            xt_load(1)
            xn_load(0)
            xt_load(2)
            xn_load(1)
            xt_load(3)
            xn_load(2)
            xn_load(3)

            madds = {}
            sc_tiles = {}

            w3 = w_sb[:].rearrange("p (c j) -> p c j", c=2)
            xt4 = xt_sb[:].rearrange("p (e c t) -> p e c t", e=BE, c=2)

            def main_tile(e, i):
                # S^T tile: partition = j (2 blocks of 128), free = t (512).
                s_ps = sps.tile([128, 2 * TT], F32)
                for jb in range(2):
                    if USE_FP8:
                        # Both K-subtiles (h chunks) in one DoubleRow matmul.
                        nc.tensor.matmul(
                            s_ps[:, jb * TT : (jb + 1) * TT],
                            w3[:, :, jb * 128 : (jb + 1) * 128],
                            xt4[:, e, :, i * TT : (i + 1) * TT],
                            start=True,
                            stop=True,
                            perf_mode=mybir.MatmulPerfMode.DoubleRow,
                        )
                    else:
                        for hc in range(2):
                            nc.tensor.matmul(
                                s_ps[:, jb * TT : (jb + 1) * TT],
                                w_sb[:, hc * H + jb * 128 : hc * H + (jb + 1) * 128],
                                xt_sb[:, (e * 2 + hc) * T + i * TT : (e * 2 + hc) * T + (i + 1) * TT],
                                start=(hc == 0),
                                stop=(hc == 1),
                            )
                sq = sqp.tile([128, 2 * TT], BF16)
                if has_bias:
                    for jb in range(2):
                        nc.scalar.activation(
                            sq[:, jb * TT : (jb + 1) * TT],
                            s_ps[:, jb * TT : (jb + 1) * TT],
                            Tanh,
                            bias=bias_sb[:, jb : jb + 1],
                        )
                else:
                    nc.scalar.activation(sq[:], s_ps[:], Tanh)

                # madd[j, t] = sq0*proj0 + sq1*proj1  (per-partition scalars, 4x mode)
                m0 = scrp.tile([128, TT], BF16, tag="m0")
                madd = scrp.tile([128, TT], BF16, tag="madd")
                nc.vector.tensor_scalar_mul(m0[:], sq[:, 0:TT], pj_sb[:, 0:1])
                nc.vector.tensor_scalar_mul(madd[:], sq[:, TT : 2 * TT], pj_sb[:, 1:2])
                nc.vector.tensor_add(madd[:], madd[:], m0[:])
                madds[(e, i)] = madd

            def scores_mm(e, i):
                # scores[t] = sum_j madd[j, t] via PE: 4 chunks of 128 t-cols,
                # accumulated straight into the example's PSUM score tile.
                madd = madds.pop((e, i))
                if i == 0:
                    sc_tiles[e] = scps.tile([128, CPE], F32, name="sc", tag="sc")
                sc_ps = sc_tiles[e]
                for c in range(SUB):
                    nc.tensor.matmul(
                        sc_ps[:, i * SUB + c : i * SUB + c + 1],
                        madd[:, c * 128 : (c + 1) * 128],
                        ones_bf[:],
                        start=True,
                        stop=True,
                    )

            def final_head_full(e, y_tiles):
                # Full per-example head: exp+mask all chunks, rowsum,
                # denominator matmul, reciprocal. Runs once, early, so the
                # y-matmuls can spread evenly over the next example's tiles.
                c0, c1 = e * CPE, (e + 1) * CPE
                e_bf = exp_pool.tile([128, CPE], BF16, name="e_bf", tag="e_bf")
                sc_ps = sc_tiles.pop(e)
                nc.scalar.activation(e_bf[:], sc_ps[:], Exp, scale=1.0 / 16.0)
                nc.vector.tensor_mul(e_m[:, c0:c1], e_bf[:], mt_sb[:, c0:c1])
                rowsum = exp_pool.tile([128, 1], F32)
                nc.vector.reduce_sum(
                    out=rowsum[:], in_=e_m[:, c0:c1], axis=mybir.AxisListType.X
                )
                s1 = ssum.tile([1, 1], F32)
                nc.tensor.matmul(s1[:], rowsum[:], ones[:], start=True, stop=True)
                sinv = exp_pool.tile([1, 1], F32)
                nc.vector.reciprocal(sinv[:], s1[:])
                y_ps = yps.tile([1, H], F32)
                y_tiles[e] = (y_ps, sinv)

            def final_mms_sp(e, y_tiles, glo, ghi):
                y_ps, _ = y_tiles[e]
                for g in range(glo, ghi):
                    gg = e * CPE + g
                    nc.tensor.matmul(
                        y_ps[:],
                        e_m[:, gg : gg + 1],
                        xn_sb[:, gg * H : (gg + 1) * H],
                        start=(g == 0),
                        stop=(g == CPE - 1),
                        skip_group_check=True,
                    )

            def final_fin(e, y_tiles):
                y_ps, sinv = y_tiles.pop(e)
                nc.vector.tensor_scalar_mul(y_sb[:, e * H : (e + 1) * H], y_ps[:], sinv[:])
                nc.sync.dma_start(y_d.ap()[e : e + 1, :], y_sb[:, e * H : (e + 1) * H])

            def final_head(e, y_tiles, ghi):
                # exp/mask for score cols [0, ghi) + y psum allocation.
                c0 = e * CPE
                e_bf = exp_pool.tile([128, CPE], BF16, name="e_bf", tag="e_bf")
                sc_ps = sc_tiles[e]
                nc.scalar.activation(
                    e_bf[:, 0:ghi], sc_ps[:, 0:ghi], Exp, scale=1.0 / 16.0
                )
                nc.vector.tensor_mul(
                    e_m[:, c0 : c0 + ghi], e_bf[:, 0:ghi], mt_sb[:, c0 : c0 + ghi]
                )
                y_ps = yps.tile([1, H], F32)
                y_tiles[e] = (y_ps, e_bf)

            def final_mms(e, y_tiles, glo, ghi):
                y_ps, _ = y_tiles[e]
                for g in range(glo, ghi):
                    gg = e * CPE + g
                    nc.tensor.matmul(
                        y_ps[:],
                        e_m[:, gg : gg + 1],
                        xn_sb[:, gg * H : (gg + 1) * H],
                        start=(g == 0),
                        stop=False,
                        skip_group_check=True,
                    )

            def final_tail(e, y_tiles, glo):
                c0, c1 = e * CPE, (e + 1) * CPE
                y_ps, e_bf = y_tiles.pop(e)
                sc_ps = sc_tiles.pop(e)
                if glo < CPE:
                    nc.scalar.activation(
                        e_bf[:, glo:CPE], sc_ps[:, glo:CPE], Exp, scale=1.0 / 16.0
                    )
                    nc.vector.tensor_mul(
                        e_m[:, c0 + glo : c1], e_bf[:, glo:CPE], mt_sb[:, c0 + glo : c1]
                    )
                rowsum = exp_pool.tile([128, 1], F32)
                nc.vector.reduce_sum(
                    out=rowsum[:], in_=e_m[:, c0:c1], axis=mybir.AxisListType.X
                )
                s1 = ssum.tile([1, 1], F32)
                nc.tensor.matmul(s1[:], rowsum[:], ones[:], start=True, stop=True)
                sinv = exp_pool.tile([1, 1], F32)
                nc.vector.reciprocal(sinv[:], s1[:])
                gfrom = min(glo, CPE - 1)
                for g in range(gfrom, CPE):
                    gg = e * CPE + g
                    nc.tensor.matmul(
                        y_ps[:],
                        e_m[:, gg : gg + 1],
                        xn_sb[:, gg * H : (gg + 1) * H],
                        start=False,
                        stop=(g == CPE - 1),
                        skip_group_check=True,
                    )
                nc.vector.tensor_scalar_mul(y_sb[:, e * H : (e + 1) * H], y_ps[:], sinv[:])
                nc.sync.dma_start(y_d.ap()[e : e + 1, :], y_sb[:, e * H : (e + 1) * H])

            def final_ex(e, y_tiles):
                final_head(e, y_tiles, CPE)
                final_tail(e, y_tiles, CPE)

            # Software-pipelined emission: scores for tile k are emitted two
            # tiles later so PE never stalls on ACT/DVE of tile k. The
            # previous example's weighted-sum matmuls spread evenly over
            # tiles i=2..7 so no tile is PE-bound.
            pending = []
            y_tiles = {}
            for e in range(BE):
                for i in range(NTILE):
                    main_tile(e, i)
                    pending.append((e, i))
                    if len(pending) > 2:
                        scores_mm(*pending.pop(0))
                    if e > 0:
                        if i == 2:
                            final_head_full(e - 1, y_tiles)
                            final_mms_sp(e - 1, y_tiles, 0, 2)
                        elif 3 <= i <= 7:
                            final_mms_sp(
                                e - 1, y_tiles, 2 + (i - 3) * 6, 2 + (i - 2) * 6
                            )
                        if i == 7:
                            final_fin(e - 1, y_tiles)
            # Last example: overlap 7 tiles' weighted sums with the last
            # tile's score chain, then finish with the final 4 chunks.
            eL = BE - 1
            scores_mm(*pending.pop(0))
            final_head(eL, y_tiles, (NTILE - 1) * SUB)
            final_mms(eL, y_tiles, 0, (NTILE - 1) * SUB)
            scores_mm(*pending.pop(0))
            final_tail(eL, y_tiles, (NTILE - 1) * SUB)

    nc.compile()
    return nc


def _get(has_bias: bool):
    if has_bias not in _compiled:
        _compiled[has_bias] = _build(has_bias)
    return _compiled[has_bias]


def make_in_maps(xes, mask, weight, bias, proj):
    xes = np.asarray(xes, dtype=np.float32)
    mask = np.asarray(mask)
    weight = np.asarray(weight, dtype=np.float32)
    bias = np.asarray(bias, dtype=np.float32)
    proj = np.asarray(proj, dtype=np.float32)

    has_bias = bool(np.any(bias))
    w2 = np.ascontiguousarray(weight.astype(fp8 if USE_FP8 else bf16).reshape(2, 128, H))
    pj = np.ascontiguousarray(proj[:, 0].astype(np.float32).reshape(2, 128).T)
    in_maps = []
    for c in range(NCORES):
        xs = xes[c * BE : (c + 1) * BE]
        xn = np.ascontiguousarray(
            xs.astype(bf16).reshape(NCH, 128, H).transpose(1, 0, 2)
        ).reshape(128, NCH * H)
        xdt = fp8 if USE_FP8 else bf16
        xt = np.ascontiguousarray(xs.transpose(0, 2, 1)).astype(xdt).reshape(
            BE * 2, 128, T
        )
        mt = np.ascontiguousarray(
            mask[c * BE : (c + 1) * BE]
            .astype(np.float32)
            .reshape(BE, CPE, 128)
            .transpose(2, 0, 1)
            .reshape(128, NCH)
        ).astype(bf16)
        m = {"xn": xn, "xt": xt, "w": w2, "pj": pj, "mt": mt}
        if has_bias:
            m["bias"] = np.ascontiguousarray(bias.reshape(2, 128).T)
        in_maps.append(m)
    return in_maps, has_bias


def _run(in_maps, has_bias, **kwargs):
    nc = _get(has_bias)
    return bass_utils.run_bass_kernel_spmd(
        nc, in_maps, core_ids=list(range(NCORES)), **kwargs
    )


def kernel(xes, mask, weight, bias, proj):
    in_maps, has_bias = make_in_maps(xes, mask, weight, bias, proj)
    res = _run(in_maps, has_bias)
    return np.concatenate([res.results[c]["y"] for c in range(NCORES)], axis=0)

